# revision 26
# baseline (speedup 1.0000x reference)
"""Multi-head attention (B=4, S=2048, D=512, H=8) on 8 TRN2 NeuronCores.

Sharding: (batch, head-group) -> core.  Core c handles batch c//2 and the
4-head group c%2; it computes those heads' contribution [2048, 512] to its
batch's output.  The host sums the two partials per batch and adds b_o.

Per-core pipeline (layouts chosen so no on-device transposes are needed):
  qkT  [ch, tok]   = W_qk @ x^T    (channels on partitions)
  v    [tok, ch|1] = x @ W_v^T     (tokens on partitions, ones column)
  logitsT [k, q]   = matmul(lhsT=kT block, rhs=qT)   (f32 PSUM)
  expT  = exp(logitsT / 8)  (ScalarE reads PSUM, writes bf16 SBUF)
  attnT+denom [65, q] = matmul(lhsT=[V | 1], rhs=expT), accumulated over k
  attn_norm = attnT * bcast(1/denom)   (DVE recip + GpSimd partition bcast)
  out [q, o] = sum_h matmul(lhsT=attn_norm_h, rhs=W_o_h)
All matmuls run in bf16 (inputs rounded on host); PSUM accumulation is f32.
"""

import numpy as np
import ml_dtypes
import concourse.bass as bass
import concourse.mybir as mybir
import concourse.tile as tile
from concourse import bacc
from concourse.bass_utils import run_bass_kernel_spmd

F32 = mybir.dt.float32
BF16 = mybir.dt.bfloat16

D = 512
DK = 64
B = 4
S = 2048
N_CORES = 8
NH = 4  # heads per core
P = 128
QC = 512  # q chunk width in main loop
KB = S // P  # 16 k blocks
TC = S // 512  # token chunks for projections

_CACHED_NC = None


def _build_nc():
    nc = bacc.Bacc("TRN2", target_bir_lowering=False, debug=False, num_devices=N_CORES)

    xT = nc.dram_tensor("xT", [D, S], BF16, kind="ExternalInput").ap()
    wqkT = nc.dram_tensor("wqkT", [D, 512], BF16, kind="ExternalInput").ap()
    wvT = nc.dram_tensor("wvT", [D, 256], BF16, kind="ExternalInput").ap()
    wo = nc.dram_tensor("wo", [NH, DK, D], BF16, kind="ExternalInput").ap()
    bqk = nc.dram_tensor("bqk", [512], F32, kind="ExternalInput").ap()
    bv = nc.dram_tensor("bv", [DK, NH], F32, kind="ExternalInput").ap()
    ones_in = nc.dram_tensor("ones_in", [P, P], BF16, kind="ExternalInput").ap()
    out = nc.dram_tensor("out", [S, D], F32, kind="ExternalOutput").ap()

    Exp = mybir.ActivationFunctionType.Exp

    with tile.TileContext(nc) as tc:
        with (
            tc.tile_pool(name="const", bufs=1) as cpool,
            tc.tile_pool(name="big", bufs=1) as bigpool,
            tc.tile_pool(name="exp", bufs=8) as epool,
            tc.tile_pool(name="raw", bufs=3) as rawpool,
            tc.tile_pool(name="rec", bufs=2) as rpool,
            tc.tile_pool(name="rbc", bufs=2) as rbcpool,
            tc.tile_pool(name="outsb", bufs=3) as opool,
            tc.tile_pool(name="lg", bufs=2, space="PSUM") as lpool,
            tc.tile_pool(name="at", bufs=2, space="PSUM") as apool,
            tc.tile_pool(name="op", bufs=2, space="PSUM") as oppool,
        ):
            # ---- load inputs -------------------------------------------------
            xT_sb = bigpool.tile([P, 4, S], BF16, tag="xT")
            xT_r = xT.rearrange("(o p) t -> p o t", p=P)
            for kb in range(4):
                nc.sync.dma_start(xT_sb[:, kb, :], xT_r[:, kb, :])
            wqk_sb = cpool.tile([P, 4, 512], BF16, tag="wqk")
            nc.sync.dma_start(wqk_sb[:], wqkT.rearrange("(o p) c -> p o c", p=P))
            wv_sb = cpool.tile([P, 4, 256], BF16, tag="wv")
            nc.sync.dma_start(wv_sb[:], wvT.rearrange("(o p) c -> p o c", p=P))
            wo_sb = cpool.tile([DK, NH, D], BF16, tag="wo")
            nc.sync.dma_start(wo_sb[:], wo.rearrange("h c o -> c h o"))
            bqk_sb = cpool.tile([P, 4], F32, tag="bqk")
            nc.sync.dma_start(bqk_sb[:], bqk.rearrange("(o p) -> p o", p=P))
            bv_sb = cpool.tile([DK, NH], F32, tag="bv")
            nc.sync.dma_start(bv_sb[:], bv[:])
            ones_sb = cpool.tile([P, P], BF16, tag="ones")
            nc.sync.dma_start(ones_sb[:], ones_in[:])
            ones_tok = ones_sb[0:1, 0:P]

            # persistent big tiles
            qkT_sb = bigpool.tile([P, 4, S], BF16, tag="qkT")
            v_sb = bigpool.tile([P, KB, NH, DK + 1], BF16, tag="v")
            attn_sb = bigpool.tile([DK, NH, S], BF16, tag="attn")

            # ones column of v via strided DMA from the host ones tensor
            nc.sync.dma_start(
                v_sb[:, :, :, DK : DK + 1],
                ones_in[:, 0:64].rearrange("p (a b c) -> p a b c", a=KB, b=NH),
            )

            # ---- qk projection: qkT[ch, tok] --------------------------------
            for chb in range(4):
                for t in range(TC):
                    ps = lpool.tile([P, 512], F32, tag="lg")
                    for kb in range(4):
                        nc.tensor.matmul(
                            ps[:],
                            wqk_sb[:, kb, chb * P : (chb + 1) * P],
                            xT_sb[:, kb, t * 512 : (t + 1) * 512],
                            start=(kb == 0),
                            stop=(kb == 3),
                        )
                    nc.vector.tensor_scalar_add(
                        qkT_sb[:, chb, t * 512 : (t + 1) * 512],
                        ps[:],
                        bqk_sb[:, chb : chb + 1],
                    )

            # ---- v projection: v[tok, ch] (+ ones col) ----------------------
            for tb in range(KB):
                ps = lpool.tile([P, 256], F32, tag="lg")
                for kb in range(4):
                    nc.tensor.matmul(
                        ps[:],
                        xT_sb[:, kb, tb * P : (tb + 1) * P],
                        wv_sb[:, kb, :],
                        start=(kb == 0),
                        stop=(kb == 3),
                    )
                nc.vector.tensor_copy(
                    v_sb[:, tb, :, 0:DK], ps[:].rearrange("p (h d) -> p h d", h=NH)
                )

            # ---- attention main loop ---------------------------------------
            for pr in range(2):  # head pair (2*pr, 2*pr+1)
                hA, hB = 2 * pr, 2 * pr + 1
                for qc in range(S // QC):
                    q0 = qc * QC
                    atA = apool.tile([DK + 1, QC], F32, tag="attn")
                    atB = apool.tile([DK + 1, QC], F32, tag="attn")
                    for kb in range(KB):
                        k0 = kb * P
                        lg = lpool.tile([P, 2, QC], F32, tag="lg")
                        nc.tensor.matmul(
                            lg[:, 0, :],
                            qkT_sb[0:DK, 2 + pr, k0 : k0 + P],
                            qkT_sb[0:DK, pr, q0 : q0 + QC],
                            start=True,
                            stop=True,
                        )
                        nc.tensor.matmul(
                            lg[:, 1, :],
                            qkT_sb[DK:P, 2 + pr, k0 : k0 + P],
                            qkT_sb[DK:P, pr, q0 : q0 + QC],
                            start=True,
                            stop=True,
                        )
                        e = epool.tile([P, 2, QC], BF16, tag="exp")
                        nc.scalar.activation(
                            e[:].rearrange("p a q -> p (a q)"),
                            lg[:].rearrange("p a q -> p (a q)"),
                            Exp,
                            scale=0.125,
                        )
                        nc.tensor.matmul(
                            atA[:],
                            v_sb[:, kb, hA, :],
                            e[:, 0, :],
                            start=(kb == 0),
                            stop=(kb == KB - 1),
                        )
                        nc.tensor.matmul(
                            atB[:],
                            v_sb[:, kb, hB, :],
                            e[:, 1, :],
                            start=(kb == 0),
                            stop=(kb == KB - 1),
                        )
                    # normalize: attn_norm = attn[0:64] * bcast(1/denom)
                    for h, at in ((hA, atA), (hB, atB)):
                        raw = rawpool.tile([DK, QC], F32, tag="raw")
                        nc.vector.tensor_copy(raw[:], at[0:DK, :])
                        den = rpool.tile([1, QC], F32, tag="den")
                        nc.vector.tensor_copy(den[:], at[DK : DK + 1, :])
                        rec1 = rpool.tile([1, QC], F32, tag="rec")
                        nc.vector.reciprocal_approx_fast(rec1[:], den[:])
                        rbc = rbcpool.tile([DK, QC], F32, tag="rbc")
                        nc.gpsimd.partition_broadcast(rbc[:], rec1[:])
                        nc.vector.tensor_tensor(
                            raw[:],
                            raw[:],
                            rbc[:],
                            mybir.AluOpType.mult,
                        )
                        nc.vector.tensor_scalar_add(
                            attn_sb[:, h, q0 : q0 + QC],
                            raw[:],
                            bv_sb[:, h : h + 1],
                        )



            # ---- output projection: out[q, o] -------------------------------
            for qb in range(S // P):
                po = oppool.tile([P, D], F32, tag="op")
                for h in range(NH):
                    nc.tensor.matmul(
                        po[:],
                        attn_sb[:, h, qb * P : (qb + 1) * P],
                        wo_sb[:, h, :],
                        start=(h == 0),
                        stop=(h == NH - 1),
                    )
                ob = opool.tile([P, D], F32, tag="ob")
                nc.vector.tensor_copy(ob[:], po[:])
                nc.sync.dma_start(out[qb * P : (qb + 1) * P, :], ob[:])

    nc.compile()
    return nc


def _get_nc():
    global _CACHED_NC
    if _CACHED_NC is None:
        _CACHED_NC = _build_nc()
    return _CACHED_NC


def run_cores(x, w_qkv, b_qkv, w_o, trace=False, **trace_kwargs):
    nc = _get_nc()
    bf = ml_dtypes.bfloat16
    in_maps = []
    for c in range(N_CORES):
        b, g = divmod(c, 2)
        heads = range(NH * g, NH * g + NH)
        q_rows = np.concatenate([w_qkv[h * 192 : h * 192 + 64] for h in heads], 0)
        k_rows = np.concatenate([w_qkv[h * 192 + 64 : h * 192 + 128] for h in heads], 0)
        v_rows = np.concatenate([w_qkv[h * 192 + 128 : h * 192 + 192] for h in heads], 0)
        bq = np.concatenate([b_qkv[h * 192 : h * 192 + 64] for h in heads], 0)
        bk = np.concatenate([b_qkv[h * 192 + 64 : h * 192 + 128] for h in heads], 0)
        bvv = np.concatenate([b_qkv[h * 192 + 128 : h * 192 + 192] for h in heads], 0)
        wo_slice = w_o[:, 256 * g : 256 * g + 256].T.reshape(NH, DK, D)
        in_maps.append(
            {
                "xT": np.ascontiguousarray(x[b].T).astype(bf),
                "wqkT": np.ascontiguousarray(
                    np.concatenate([q_rows, k_rows], 0).T
                ).astype(bf),
                "wvT": np.ascontiguousarray(v_rows.T).astype(bf),
                "wo": np.ascontiguousarray(wo_slice).astype(bf),
                "bqk": np.ascontiguousarray(np.concatenate([bq, bk], 0)).astype(
                    np.float32
                ),
                "bv": np.ascontiguousarray(bvv.reshape(NH, DK).T).astype(np.float32),
                "ones_in": np.ones((P, P), bf),
            }
        )
    res = run_bass_kernel_spmd(
        nc, in_maps, core_ids=list(range(N_CORES)), trace=trace, **trace_kwargs
    )
    return res


def kernel(x, w_qkv, b_qkv, w_o, b_o):
    x = np.asarray(x, dtype=np.float32)
    w_qkv = np.asarray(w_qkv, dtype=np.float32)
    b_qkv = np.asarray(b_qkv, dtype=np.float32)
    w_o = np.asarray(w_o, dtype=np.float32)
    b_o = np.asarray(b_o, dtype=np.float32)
    res = run_cores(x, w_qkv, b_qkv, w_o)
    out = np.empty((B, S, D), np.float32)
    for b in range(B):
        out[b] = res.results[2 * b]["out"] + res.results[2 * b + 1]["out"] + b_o
    return out


# revision 27
# speedup vs baseline: 1.0593x; 1.0593x over previous
"""Multi-head attention (B=4, S=2048, D=512, H=8) on 8 TRN2 NeuronCores.

Sharding: (batch, head-group) -> core.  Core c handles batch c//2 and the
4-head group c%2; it computes those heads' contribution [2048, 512] to its
batch's output.  The host sums the two partials per batch and adds b_o.

Per-core pipeline (layouts chosen so no on-device transposes are needed):
  qkT  [ch, tok]   = W_qk @ x^T    (channels on partitions)
  v    [tok, ch|1] = x @ W_v^T     (tokens on partitions, ones column)
  logitsT [k, q]   = matmul(lhsT=kT block, rhs=qT)   (f32 PSUM)
  expT  = exp(logitsT / 8)  (ScalarE reads PSUM, writes bf16 SBUF)
  attnT+denom [65, q] = matmul(lhsT=[V | 1], rhs=expT), accumulated over k
  attn_norm = attnT * bcast(1/denom)   (DVE recip + GpSimd partition bcast)
  out [q, o] = sum_h matmul(lhsT=attn_norm_h, rhs=W_o_h)
All matmuls run in bf16 (inputs rounded on host); PSUM accumulation is f32.
"""

import numpy as np
import ml_dtypes
import concourse.bass as bass
import concourse.mybir as mybir
import concourse.tile as tile
from concourse import bacc
from concourse.bass_utils import run_bass_kernel_spmd

F32 = mybir.dt.float32
BF16 = mybir.dt.bfloat16

D = 512
DK = 64
B = 4
S = 2048
N_CORES = 8
NH = 4  # heads per core
P = 128
QC = 512  # q chunk width in main loop
KB = S // P  # 16 k blocks
TC = S // 512  # token chunks for projections

_CACHED_NC = None


def _build_nc():
    nc = bacc.Bacc("TRN2", target_bir_lowering=False, debug=False, num_devices=N_CORES)

    xT = nc.dram_tensor("xT", [D, S], BF16, kind="ExternalInput").ap()
    wqkT = nc.dram_tensor("wqkT", [D, 512], BF16, kind="ExternalInput").ap()
    wvT = nc.dram_tensor("wvT", [D, 256], BF16, kind="ExternalInput").ap()
    wo = nc.dram_tensor("wo", [2, P, D], BF16, kind="ExternalInput").ap()
    bqk = nc.dram_tensor("bqk", [512], F32, kind="ExternalInput").ap()
    bv = nc.dram_tensor("bv", [DK, NH], F32, kind="ExternalInput").ap()
    ones_in = nc.dram_tensor("ones_in", [P, P], BF16, kind="ExternalInput").ap()
    out = nc.dram_tensor("out", [S, D], F32, kind="ExternalOutput").ap()

    Exp = mybir.ActivationFunctionType.Exp

    with tile.TileContext(nc) as tc:
        with (
            tc.tile_pool(name="const", bufs=1) as cpool,
            tc.tile_pool(name="big", bufs=1) as bigpool,
            tc.tile_pool(name="exp", bufs=8) as epool,
            tc.tile_pool(name="raw", bufs=3) as rawpool,
            tc.tile_pool(name="rec", bufs=2) as rpool,
            tc.tile_pool(name="rbc", bufs=2) as rbcpool,
            tc.tile_pool(name="outsb", bufs=3) as opool,
            tc.tile_pool(name="lg", bufs=3, space="PSUM") as lpool,
            tc.tile_pool(name="at", bufs=2, space="PSUM") as apool,
        ):
            # ---- load inputs -------------------------------------------------
            xT_sb = bigpool.tile([P, 4, S], BF16, tag="xT")
            xT_r = xT.rearrange("(o p) t -> p o t", p=P)
            for kb in range(4):
                nc.sync.dma_start(xT_sb[:, kb, :], xT_r[:, kb, :])
            wqk_sb = cpool.tile([P, 4, 512], BF16, tag="wqk")
            nc.sync.dma_start(wqk_sb[:], wqkT.rearrange("(o p) c -> p o c", p=P))
            wv_sb = cpool.tile([P, 4, 256], BF16, tag="wv")
            nc.sync.dma_start(wv_sb[:], wvT.rearrange("(o p) c -> p o c", p=P))
            wo_sb = cpool.tile([P, 2, D], BF16, tag="wo")
            nc.sync.dma_start(wo_sb[:], wo.rearrange("a p o -> p a o"))
            bqk_sb = cpool.tile([P, 4], F32, tag="bqk")
            nc.sync.dma_start(bqk_sb[:], bqk.rearrange("(o p) -> p o", p=P))
            bv_sb = cpool.tile([DK, NH], F32, tag="bv")
            nc.sync.dma_start(bv_sb[:], bv[:])
            ones_sb = cpool.tile([P, P], BF16, tag="ones")
            nc.sync.dma_start(ones_sb[:], ones_in[:])
            ones_tok = ones_sb[0:1, 0:P]

            # persistent big tiles
            qkT_sb = bigpool.tile([P, 4, S], BF16, tag="qkT")
            v_sb = bigpool.tile([P, KB, NH, DK + 1], BF16, tag="v")
            attn_sb = bigpool.tile([P, 2, S], BF16, tag="attn")

            # ones column of v via strided DMA from the host ones tensor
            nc.sync.dma_start(
                v_sb[:, :, :, DK : DK + 1],
                ones_in[:, 0:64].rearrange("p (a b c) -> p a b c", a=KB, b=NH),
            )

            # ---- qk projection: qkT[ch, tok] --------------------------------
            for chb in range(4):
                for t in range(TC):
                    ps = lpool.tile([P, 512], F32, tag="lg")
                    for kb in range(4):
                        nc.tensor.matmul(
                            ps[:],
                            wqk_sb[:, kb, chb * P : (chb + 1) * P],
                            xT_sb[:, kb, t * 512 : (t + 1) * 512],
                            start=(kb == 0),
                            stop=(kb == 3),
                        )
                    nc.vector.tensor_scalar_add(
                        qkT_sb[:, chb, t * 512 : (t + 1) * 512],
                        ps[:],
                        bqk_sb[:, chb : chb + 1],
                    )

            # ---- v projection: v[tok, ch] (+ ones col) ----------------------
            for tb in range(KB):
                ps = lpool.tile([P, 256], F32, tag="lg")
                for kb in range(4):
                    nc.tensor.matmul(
                        ps[:],
                        xT_sb[:, kb, tb * P : (tb + 1) * P],
                        wv_sb[:, kb, :],
                        start=(kb == 0),
                        stop=(kb == 3),
                    )
                nc.vector.tensor_copy(
                    v_sb[:, tb, :, 0:DK], ps[:].rearrange("p (h d) -> p h d", h=NH)
                )

            # ---- attention main loop ---------------------------------------
            for pr in range(2):  # head pair (2*pr, 2*pr+1)
                hA, hB = 2 * pr, 2 * pr + 1
                for qc in range(S // QC):
                    q0 = qc * QC
                    atA = apool.tile([DK + 1, QC], F32, tag="attn")
                    atB = apool.tile([DK + 1, QC], F32, tag="attn")
                    for kb in range(KB):
                        k0 = kb * P
                        lg = lpool.tile([P, 2, QC], F32, tag="lg")
                        nc.tensor.matmul(
                            lg[:, 0, :],
                            qkT_sb[0:DK, 2 + pr, k0 : k0 + P],
                            qkT_sb[0:DK, pr, q0 : q0 + QC],
                            start=True,
                            stop=True,
                        )
                        nc.tensor.matmul(
                            lg[:, 1, :],
                            qkT_sb[DK:P, 2 + pr, k0 : k0 + P],
                            qkT_sb[DK:P, pr, q0 : q0 + QC],
                            start=True,
                            stop=True,
                        )
                        e = epool.tile([P, 2, QC], BF16, tag="exp")
                        nc.scalar.activation(
                            e[:].rearrange("p a q -> p (a q)"),
                            lg[:].rearrange("p a q -> p (a q)"),
                            Exp,
                            scale=0.125,
                        )
                        nc.tensor.matmul(
                            atA[:],
                            v_sb[:, kb, hA, :],
                            e[:, 0, :],
                            start=(kb == 0),
                            stop=(kb == KB - 1),
                        )
                        nc.tensor.matmul(
                            atB[:],
                            v_sb[:, kb, hB, :],
                            e[:, 1, :],
                            start=(kb == 0),
                            stop=(kb == KB - 1),
                        )
                    # normalize: attn_norm = attn[0:64] * bcast(1/denom)
                    for h, at in ((hA, atA), (hB, atB)):
                        raw = rawpool.tile([DK, QC], F32, tag="raw")
                        nc.vector.tensor_copy(raw[:], at[0:DK, :])
                        den = rpool.tile([1, QC], F32, tag="den")
                        nc.vector.tensor_copy(den[:], at[DK : DK + 1, :])
                        rec1 = rpool.tile([1, QC], F32, tag="rec")
                        nc.vector.reciprocal_approx_fast(rec1[:], den[:])
                        rbc = rbcpool.tile([DK, QC], F32, tag="rbc")
                        nc.gpsimd.partition_broadcast(rbc[:], rec1[:])
                        nc.vector.tensor_tensor(
                            raw[:],
                            raw[:],
                            rbc[:],
                            mybir.AluOpType.mult,
                        )
                        half = 64 * (h % 2)
                        nc.vector.tensor_scalar_add(
                            attn_sb[half : half + DK, h // 2, q0 : q0 + QC],
                            raw[:],
                            bv_sb[:, h : h + 1],
                        )



            # ---- output projection: out[q, o] -------------------------------
            for qb in range(S // P):
                po = lpool.tile([P, D], F32, tag="lg")
                for prj in range(2):
                    nc.tensor.matmul(
                        po[:],
                        attn_sb[:, prj, qb * P : (qb + 1) * P],
                        wo_sb[:, prj, :],
                        start=(prj == 0),
                        stop=(prj == 1),
                    )
                ob = opool.tile([P, D], F32, tag="ob")
                nc.vector.tensor_copy(ob[:], po[:])
                nc.sync.dma_start(out[qb * P : (qb + 1) * P, :], ob[:])

    nc.compile()
    return nc


def _get_nc():
    global _CACHED_NC
    if _CACHED_NC is None:
        _CACHED_NC = _build_nc()
    return _CACHED_NC


def run_cores(x, w_qkv, b_qkv, w_o, trace=False, **trace_kwargs):
    nc = _get_nc()
    bf = ml_dtypes.bfloat16
    in_maps = []
    for c in range(N_CORES):
        b, g = divmod(c, 2)
        heads = range(NH * g, NH * g + NH)
        q_rows = np.concatenate([w_qkv[h * 192 : h * 192 + 64] for h in heads], 0)
        k_rows = np.concatenate([w_qkv[h * 192 + 64 : h * 192 + 128] for h in heads], 0)
        v_rows = np.concatenate([w_qkv[h * 192 + 128 : h * 192 + 192] for h in heads], 0)
        bq = np.concatenate([b_qkv[h * 192 : h * 192 + 64] for h in heads], 0)
        bk = np.concatenate([b_qkv[h * 192 + 64 : h * 192 + 128] for h in heads], 0)
        bvv = np.concatenate([b_qkv[h * 192 + 128 : h * 192 + 192] for h in heads], 0)
        wo_slice = w_o[:, 256 * g : 256 * g + 256].T.reshape(2, P, D)
        in_maps.append(
            {
                "xT": np.ascontiguousarray(x[b].T).astype(bf),
                "wqkT": np.ascontiguousarray(
                    np.concatenate([q_rows, k_rows], 0).T
                ).astype(bf),
                "wvT": np.ascontiguousarray(v_rows.T).astype(bf),
                "wo": np.ascontiguousarray(wo_slice).astype(bf),
                "bqk": np.ascontiguousarray(np.concatenate([bq, bk], 0)).astype(
                    np.float32
                ),
                "bv": np.ascontiguousarray(bvv.reshape(NH, DK).T).astype(np.float32),
                "ones_in": np.ones((P, P), bf),
            }
        )
    res = run_bass_kernel_spmd(
        nc, in_maps, core_ids=list(range(N_CORES)), trace=trace, **trace_kwargs
    )
    return res


def kernel(x, w_qkv, b_qkv, w_o, b_o):
    x = np.asarray(x, dtype=np.float32)
    w_qkv = np.asarray(w_qkv, dtype=np.float32)
    b_qkv = np.asarray(b_qkv, dtype=np.float32)
    w_o = np.asarray(w_o, dtype=np.float32)
    b_o = np.asarray(b_o, dtype=np.float32)
    res = run_cores(x, w_qkv, b_qkv, w_o)
    out = np.empty((B, S, D), np.float32)
    for b in range(B):
        out[b] = res.results[2 * b]["out"] + res.results[2 * b + 1]["out"] + b_o
    return out


# revision 28
# speedup vs baseline: 1.0713x; 1.0114x over previous
"""Multi-head attention (B=4, S=2048, D=512, H=8) on 8 TRN2 NeuronCores.

Sharding: (batch, head-group) -> core.  Core c handles batch c//2 and the
4-head group c%2; it computes those heads' contribution [2048, 512] to its
batch's output.  The host sums the two partials per batch and adds b_o.

Per-core pipeline (layouts chosen so no on-device transposes are needed):
  qkT  [ch, tok]   = W_qk @ x^T    (channels on partitions)
  v    [tok, ch|1] = x @ W_v^T     (tokens on partitions, ones column)
  logitsT [k, q]   = matmul(lhsT=kT block, rhs=qT)   (f32 PSUM)
  expT  = exp(logitsT / 8)  (ScalarE reads PSUM, writes bf16 SBUF)
  attnT+denom [65, q] = matmul(lhsT=[V | 1], rhs=expT), accumulated over k
  attn_norm = attnT * bcast(1/denom)   (DVE recip + GpSimd partition bcast)
  out [q, o] = sum_h matmul(lhsT=attn_norm_h, rhs=W_o_h)
All matmuls run in bf16 (inputs rounded on host); PSUM accumulation is f32.
"""

import numpy as np
import ml_dtypes
import concourse.bass as bass
import concourse.mybir as mybir
import concourse.tile as tile
from concourse import bacc
from concourse.bass_utils import run_bass_kernel_spmd

F32 = mybir.dt.float32
BF16 = mybir.dt.bfloat16

D = 512
DK = 64
B = 4
S = 2048
N_CORES = 8
NH = 4  # heads per core
P = 128
QC = 512  # q chunk width in main loop
KB = S // P  # 16 k blocks
TC = S // 512  # token chunks for projections

_CACHED_NC = None


def _build_nc():
    nc = bacc.Bacc("TRN2", target_bir_lowering=False, debug=False, num_devices=N_CORES)

    xT = nc.dram_tensor("xT", [D, S], BF16, kind="ExternalInput").ap()
    wqkT = nc.dram_tensor("wqkT", [D, 512], BF16, kind="ExternalInput").ap()
    wvT = nc.dram_tensor("wvT", [D, 256], BF16, kind="ExternalInput").ap()
    wo = nc.dram_tensor("wo", [2, P, D], BF16, kind="ExternalInput").ap()
    bqk = nc.dram_tensor("bqk", [512], F32, kind="ExternalInput").ap()
    bv = nc.dram_tensor("bv", [DK, NH], F32, kind="ExternalInput").ap()
    ones_in = nc.dram_tensor("ones_in", [P, P], BF16, kind="ExternalInput").ap()
    out = nc.dram_tensor("out", [S, D], F32, kind="ExternalOutput").ap()

    Exp = mybir.ActivationFunctionType.Exp

    with tile.TileContext(nc) as tc:
        with (
            tc.tile_pool(name="const", bufs=1) as cpool,
            tc.tile_pool(name="big", bufs=1) as bigpool,
            tc.tile_pool(name="exp", bufs=12) as epool,
            tc.tile_pool(name="raw", bufs=4) as rawpool,
            tc.tile_pool(name="rec", bufs=2) as rpool,
            tc.tile_pool(name="rbc", bufs=3) as rbcpool,
            tc.tile_pool(name="outsb", bufs=4) as opool,
            tc.tile_pool(name="lg", bufs=3, space="PSUM") as lpool,
            tc.tile_pool(name="at", bufs=2, space="PSUM") as apool,
        ):
            # ---- load inputs -------------------------------------------------
            xT_sb = bigpool.tile([P, 4, S], BF16, tag="xT")
            xT_r = xT.rearrange("(o p) t -> p o t", p=P)
            for kb in range(4):
                nc.sync.dma_start(xT_sb[:, kb, :], xT_r[:, kb, :])
            wqk_sb = cpool.tile([P, 4, 512], BF16, tag="wqk")
            nc.sync.dma_start(wqk_sb[:], wqkT.rearrange("(o p) c -> p o c", p=P))
            wv_sb = cpool.tile([P, 4, 256], BF16, tag="wv")
            nc.sync.dma_start(wv_sb[:], wvT.rearrange("(o p) c -> p o c", p=P))
            wo_sb = cpool.tile([P, 2, D], BF16, tag="wo")
            nc.sync.dma_start(wo_sb[:], wo.rearrange("a p o -> p a o"))
            bqk_sb = cpool.tile([P, 4], F32, tag="bqk")
            nc.sync.dma_start(bqk_sb[:], bqk.rearrange("(o p) -> p o", p=P))
            bv_sb = cpool.tile([DK, NH], F32, tag="bv")
            nc.sync.dma_start(bv_sb[:], bv[:])
            ones_sb = cpool.tile([P, P], BF16, tag="ones")
            nc.sync.dma_start(ones_sb[:], ones_in[:])
            ones_tok = ones_sb[0:1, 0:P]

            # persistent big tiles
            qkT_sb = bigpool.tile([P, 4, S], BF16, tag="qkT")
            v_sb = bigpool.tile([P, KB, NH, DK + 1], BF16, tag="v")
            attn_sb = bigpool.tile([P, 2, S], BF16, tag="attn")

            # ones column of v via strided DMA from the host ones tensor
            nc.sync.dma_start(
                v_sb[:, :, :, DK : DK + 1],
                ones_in[:, 0:64].rearrange("p (a b c) -> p a b c", a=KB, b=NH),
            )

            # ---- qk projection: qkT[ch, tok] --------------------------------
            for chb in range(4):
                for t in range(TC):
                    ps = lpool.tile([P, 512], F32, tag="lg")
                    for kb in range(4):
                        nc.tensor.matmul(
                            ps[:],
                            wqk_sb[:, kb, chb * P : (chb + 1) * P],
                            xT_sb[:, kb, t * 512 : (t + 1) * 512],
                            start=(kb == 0),
                            stop=(kb == 3),
                        )
                    nc.vector.tensor_scalar_add(
                        qkT_sb[:, chb, t * 512 : (t + 1) * 512],
                        ps[:],
                        bqk_sb[:, chb : chb + 1],
                    )

            # ---- v projection: v[tok, ch] (+ ones col) ----------------------
            for tb in range(KB):
                ps = lpool.tile([P, 256], F32, tag="lg")
                for kb in range(4):
                    nc.tensor.matmul(
                        ps[:],
                        xT_sb[:, kb, tb * P : (tb + 1) * P],
                        wv_sb[:, kb, :],
                        start=(kb == 0),
                        stop=(kb == 3),
                    )
                nc.vector.tensor_copy(
                    v_sb[:, tb, :, 0:DK], ps[:].rearrange("p (h d) -> p h d", h=NH)
                )

            # ---- attention main loop ---------------------------------------
            for pr in range(2):  # head pair (2*pr, 2*pr+1)
                hA, hB = 2 * pr, 2 * pr + 1
                for qc in range(S // QC):
                    q0 = qc * QC
                    atA = apool.tile([DK + 1, QC], F32, tag="attn")
                    atB = apool.tile([DK + 1, QC], F32, tag="attn")
                    for kb in range(KB):
                        k0 = kb * P
                        lg = lpool.tile([P, 2, QC], F32, tag="lg")
                        nc.tensor.matmul(
                            lg[:, 0, :],
                            qkT_sb[0:DK, 2 + pr, k0 : k0 + P],
                            qkT_sb[0:DK, pr, q0 : q0 + QC],
                            start=True,
                            stop=True,
                        )
                        nc.tensor.matmul(
                            lg[:, 1, :],
                            qkT_sb[DK:P, 2 + pr, k0 : k0 + P],
                            qkT_sb[DK:P, pr, q0 : q0 + QC],
                            start=True,
                            stop=True,
                        )
                        e = epool.tile([P, 2, QC], BF16, tag="exp")
                        nc.scalar.activation(
                            e[:].rearrange("p a q -> p (a q)"),
                            lg[:].rearrange("p a q -> p (a q)"),
                            Exp,
                            scale=0.125,
                        )
                        nc.tensor.matmul(
                            atA[:],
                            v_sb[:, kb, hA, :],
                            e[:, 0, :],
                            start=(kb == 0),
                            stop=(kb == KB - 1),
                        )
                        nc.tensor.matmul(
                            atB[:],
                            v_sb[:, kb, hB, :],
                            e[:, 1, :],
                            start=(kb == 0),
                            stop=(kb == KB - 1),
                        )
                    # normalize: attn_norm = attn[0:64] * bcast(1/denom)
                    for h, at in ((hA, atA), (hB, atB)):
                        raw = rawpool.tile([DK, QC], F32, tag="raw")
                        nc.vector.tensor_copy(raw[:], at[0:DK, :])
                        den = rpool.tile([1, QC], F32, tag="den")
                        nc.vector.tensor_copy(den[:], at[DK : DK + 1, :])
                        rec1 = rpool.tile([1, QC], F32, tag="rec")
                        nc.vector.reciprocal_approx_fast(rec1[:], den[:])
                        rbc = rbcpool.tile([DK, QC], F32, tag="rbc")
                        nc.gpsimd.partition_broadcast(rbc[:], rec1[:])
                        nc.vector.tensor_tensor(
                            raw[:],
                            raw[:],
                            rbc[:],
                            mybir.AluOpType.mult,
                        )
                        half = 64 * (h % 2)
                        nc.vector.tensor_scalar_add(
                            attn_sb[half : half + DK, h // 2, q0 : q0 + QC],
                            raw[:],
                            bv_sb[:, h : h + 1],
                        )



            # ---- output projection: out[q, o] -------------------------------
            for qb in range(S // P):
                po = lpool.tile([P, D], F32, tag="lg")
                for prj in range(2):
                    nc.tensor.matmul(
                        po[:],
                        attn_sb[:, prj, qb * P : (qb + 1) * P],
                        wo_sb[:, prj, :],
                        start=(prj == 0),
                        stop=(prj == 1),
                    )
                ob = opool.tile([P, D], F32, tag="ob")
                nc.vector.tensor_copy(ob[:], po[:])
                nc.sync.dma_start(out[qb * P : (qb + 1) * P, :], ob[:])

    nc.compile()
    return nc


def _get_nc():
    global _CACHED_NC
    if _CACHED_NC is None:
        _CACHED_NC = _build_nc()
    return _CACHED_NC


def run_cores(x, w_qkv, b_qkv, w_o, trace=False, **trace_kwargs):
    nc = _get_nc()
    bf = ml_dtypes.bfloat16
    in_maps = []
    for c in range(N_CORES):
        b, g = divmod(c, 2)
        heads = range(NH * g, NH * g + NH)
        q_rows = np.concatenate([w_qkv[h * 192 : h * 192 + 64] for h in heads], 0)
        k_rows = np.concatenate([w_qkv[h * 192 + 64 : h * 192 + 128] for h in heads], 0)
        v_rows = np.concatenate([w_qkv[h * 192 + 128 : h * 192 + 192] for h in heads], 0)
        bq = np.concatenate([b_qkv[h * 192 : h * 192 + 64] for h in heads], 0)
        bk = np.concatenate([b_qkv[h * 192 + 64 : h * 192 + 128] for h in heads], 0)
        bvv = np.concatenate([b_qkv[h * 192 + 128 : h * 192 + 192] for h in heads], 0)
        wo_slice = w_o[:, 256 * g : 256 * g + 256].T.reshape(2, P, D)
        in_maps.append(
            {
                "xT": np.ascontiguousarray(x[b].T).astype(bf),
                "wqkT": np.ascontiguousarray(
                    np.concatenate([q_rows, k_rows], 0).T
                ).astype(bf),
                "wvT": np.ascontiguousarray(v_rows.T).astype(bf),
                "wo": np.ascontiguousarray(wo_slice).astype(bf),
                "bqk": np.ascontiguousarray(np.concatenate([bq, bk], 0)).astype(
                    np.float32
                ),
                "bv": np.ascontiguousarray(bvv.reshape(NH, DK).T).astype(np.float32),
                "ones_in": np.ones((P, P), bf),
            }
        )
    res = run_bass_kernel_spmd(
        nc, in_maps, core_ids=list(range(N_CORES)), trace=trace, **trace_kwargs
    )
    return res


def kernel(x, w_qkv, b_qkv, w_o, b_o):
    x = np.asarray(x, dtype=np.float32)
    w_qkv = np.asarray(w_qkv, dtype=np.float32)
    b_qkv = np.asarray(b_qkv, dtype=np.float32)
    w_o = np.asarray(w_o, dtype=np.float32)
    b_o = np.asarray(b_o, dtype=np.float32)
    res = run_cores(x, w_qkv, b_qkv, w_o)
    out = np.empty((B, S, D), np.float32)
    for b in range(B):
        out[b] = res.results[2 * b]["out"] + res.results[2 * b + 1]["out"] + b_o
    return out
